# revision 17
# baseline (speedup 1.0000x reference)
"""Trainium2 Bass kernel for nn_AttentionBlock (B=4, C=256, H=W=64, IC=128).

Sharding: 8 cores = 4 batches x 2 row-halves of the N=4096 attention dim.
Each core computes its 2048 rows of the attention output, the final 1x1 conv
(wy), and partial BatchNorm statistics; a peer-to-peer SBUF exchange (or an
AllReduce fallback) combines the BN stats; each core then applies BN +
residual and writes its output slice.

Algebraic simplifications vs the reference (all exact):
  - g_b and w_b only add a per-channel constant to wy, which BatchNorm's
    mean subtraction cancels -> dropped.
  - dy_b (phi bias) only adds row-constant terms to the attention logits,
    which softmax cancels -> dropped. Only dx_b (theta bias) is applied.
  - softmax computed without max-subtraction: logits are bounded
    (|f| < ~70 for randn inputs), within bf16 exp range.
  - BN linear sums computed as wwT.T @ sum_n(y2norm) instead of summing wy.

v4 structure (209us v2 baseline -> this):
  - Softmax denominator accumulated in bf16 (2x DVE mode) with three
    accumulators: 3-of-4 adds on DVE, 1-of-4 on Pool; accumulators
    initialized by adding two exp tiles (no init copies); final
    column-reduce via 6 bf16 ones-matmuls on the PE.
  - Scalar engine runs ONLY exp in the main loop (64 x ~1.05us = floor).
  - y2 matmuls are emitted one iteration LATE (delayed-y2): every PE
    instruction's dependencies are satisfied a full iteration ahead, so
    the PE stream has no semaphore gaps and holds its full 2.4GHz p-state.
  - Projections interleaved in fine-grained blocks into the h0 loop.
  - Weights DMA'd first (200KB) so the first theta matmul isn't gated on
    the 6MB x/y stream.
  - BN stats exchanged via direct SBUF->SBUF remote DMA between the 8
    cores (XOR-rotation addressing, SPMD-safe), preceded by a sem-only
    barrier round for alignment: ~4us exposed vs ~30us for the DRAM
    collective. K_RDMA=0 falls back to a single AllReduce.
"""

import os
import sys
import numpy as np

if "/opt/trn_rl_repo" not in sys.path:
    sys.path.insert(0, "/opt/trn_rl_repo")

import concourse.bass as bass
import concourse.bacc as bacc
import concourse.mybir as mybir
import concourse.tile as tile
from concourse.bass_utils import run_bass_kernel_spmd

N_CORES = 8
B, C, HW = 4, 256, 64
N = HW * HW          # 4096 spatial positions per batch
IC = 128             # inter channels
NL = N // 2          # 2048 rows per core
NH = NL // 2         # 1024 rows per attention half
EPS = 1e-5
CNT = float(B * N)   # BatchNorm count per channel

f32 = mybir.dt.float32
f32r = mybir.dt.float32r
bf16 = mybir.dt.bfloat16
f16 = mybir.dt.float16
ALU = mybir.AluOpType
ACTF = mybir.ActivationFunctionType

DACC16 = os.environ.get("K_DACC16", "1") == "1"   # bf16 d-accumulators
RDMA = os.environ.get("K_RDMA", "0") == "1"       # p2p stats exchange


def _mm(nc, out, lhsT, rhs, start=True, stop=True):
    return nc.tensor.matmul(out, lhsT, rhs, start=start, stop=stop)


def _build():
    nc = bacc.Bacc("TRN2", target_bir_lowering=False, debug=False,
                   num_devices=N_CORES)

    xl_d = nc.dram_tensor("xl", [C, NL], f32, kind="ExternalInput").ap()
    yl_d = nc.dram_tensor("yl", [C, N], f32, kind="ExternalInput").ap()
    wpk_d = nc.dram_tensor("wpk", [C, 386], f32, kind="ExternalInput").ap()
    wpk2_d = nc.dram_tensor("wpk2", [IC, 257], f32, kind="ExternalInput").ap()
    out_d = nc.dram_tensor("out", [C, NL], f32, kind="ExternalOutput").ap()

    with tile.TileContext(nc) as tc:
        _emit(nc, tc, xl_d, yl_d, wpk_d, wpk2_d, out_d)
    nc.compile()
    return nc


def _emit(nc, tc, xl_d, yl_d, wpk_d, wpk2_d, out_d):
    DT_ACC = bf16 if DACC16 else f32r
    with (
        tc.tile_pool(name="sb_w", bufs=1) as wp,        # weights + tiny tiles
        tc.tile_pool(name="sb_x", bufs=1) as xp,        # x / y staging
        tc.tile_pool(name="sb_a", bufs=1) as ap_,       # theta/phi/g activations
        tc.tile_pool(name="sb_e", bufs=6) as ep,        # exp tiles
        tc.tile_pool(name="sb_m", bufs=2) as mp,        # misc per-half tiles
        tc.tile_pool(name="sb_bn", bufs=1) as bp,       # bn tiny tiles
        tc.tile_pool(name="ps_q", bufs=2, space="PSUM") as pq,    # ft/dq/rb
        tc.tile_pool(name="ps_a", bufs=2, space="PSUM") as pa,    # y2 h0 / wyp
        tc.tile_pool(name="ps_b", bufs=2, space="PSUM") as pb,    # proj / y2 h1
        tc.tile_pool(name="dram", bufs=1, space="DRAM") as dr,
    ):
        # ---------------- DMAs: weights first (small), then x t0, y0 -------
        w1 = [wp.tile([128, 386], f32, tag=f"w1_{i}", name=f"w1_{i}")
              for i in range(2)]
        w2 = wp.tile([IC, 257], f32, tag="w2")
        for i in range(2):
            nc.sync.dma_start(w1[i][:], wpk_d[128 * i:128 * (i + 1), :])
        nc.sync.dma_start(w2[:], wpk2_d[:])

        xl_t = [xp.tile([128, NL], f32, tag=f"xl{c}", bufs=1, name=f"xl{c}")
                for c in range(2)]
        for c in range(2):
            nc.sync.dma_start(xl_t[c][:, 0:NH], xl_d[128 * c:128 * (c + 1), 0:NH])

        yh_t = [xp.tile([128, N], f16, tag=f"yh{c}", bufs=1, name=f"yh{c}")
                for c in range(2)]

        def emit_y_dma(t):
            sl = slice(1024 * t, 1024 * (t + 1))
            for c in range(2):
                nc.gpsimd.dma_start(yh_t[c][:, sl],
                                    yl_d[128 * c:128 * (c + 1), sl])

        emit_y_dma(0)
        for c in range(2):
            nc.sync.dma_start(xl_t[c][:, NH:NL], xl_d[128 * c:128 * (c + 1), NH:NL])
        emit_y_dma(1)

        # ---------------- weight casts (DVE) ----------------
        wh1 = [wp.tile([128, 384], f16, tag=f"wh1_{i}", name=f"wh1_{i}")
               for i in range(2)]
        for i in range(2):
            nc.vector.tensor_copy(wh1[i][:], w1[i][:, 0:384])
        wdx_h = [wh1[i][:, 0:128] for i in range(2)]
        wdy_h = [wh1[i][:, 128:256] for i in range(2)]
        wg_h = [wh1[i][:, 256:384] for i in range(2)]
        gamma_t = [w1[i][:, 384:385] for i in range(2)]
        beta_t = [w1[i][:, 385:386] for i in range(2)]

        xh_t = [xp.tile([128, NL], f16, tag=f"xh{c}", bufs=1, name=f"xh{c}")
                for c in range(2)]
        for c in range(2):
            nc.vector.tensor_copy(xh_t[c][:, 0:NH], xl_t[c][:, 0:NH])

        wwT_b = wp.tile([IC, C], bf16, tag="wwT_b")
        nc.vector.tensor_copy(wwT_b[:], w2[:, 0:256])
        dxb_t = wp.tile([IC, 1], f32, tag="dxb")
        nc.vector.tensor_copy(dxb_t[:], w2[:, 256:257])

        ones_mb = wp.tile([128, 1], bf16, tag="ones_mb")  # d-reduce stationary
        nc.vector.memset(ones_mb[:], 1.0)
        ones_mf = wp.tile([128, 1], f32, tag="ones_mf")
        nc.vector.memset(ones_mf[:], 1.0)
        ones_r = wp.tile([1, 128], f32, tag="ones_r")     # rinv bcast stationary
        nc.vector.memset(ones_r[:], 1.0)

        if RDMA:
            # stats recv buffer: written ONLY by remote cores; memset early
            # so the allocator sees a writer (peers write >80us later)
            rdrecv = wp.tile([128, 56], f32, tag="rdrecv")   # 7 slots x 8
            nc.vector.memset(rdrecv[:], 0.0)
            bar_sem = nc.alloc_semaphore("bn_bar")
            st_sem = nc.alloc_semaphore("bn_st")
            loc_sem = nc.alloc_semaphore("bn_loc")

        for c in range(2):
            nc.vector.tensor_copy(xh_t[c][:, NH:NL], xl_t[c][:, NH:NL])

        # ---------------- projection targets ----------------
        theta_h = ap_.tile([IC, NL], f16, tag="theta")
        phi_h = ap_.tile([IC, N], f16, tag="phi")
        g_sb = ap_.tile([128, N], bf16, tag="g")   # 32 chunks [m128, ic128]

        def emit_theta_block(tb):  # tb 0..3 : theta n-cols 512*tb
            sl = slice(512 * tb, 512 * (tb + 1))
            tp_ = pb.tile([128, 512], f32, tag="pj", name=f"thp{tb}")
            for c in range(2):
                _mm(nc, tp_[:], wdx_h[c], xh_t[c][:, sl],
                    start=(c == 0), stop=(c == 1))
            nc.vector.tensor_scalar(theta_h[:, sl], tp_[:], dxb_t[:],
                                    None, ALU.add)

        def emit_phi_block(pbk):  # pbk 0..7 : phi m-cols 512*pbk
            sl = slice(512 * pbk, 512 * (pbk + 1))
            pp_ = pb.tile([128, 512], f32, tag="pj", name=f"php{pbk}")
            for c in range(2):
                _mm(nc, pp_[:], wdy_h[c], yh_t[c][:, sl],
                    start=(c == 0), stop=(c == 1))
            nc.vector.tensor_copy(phi_h[:, sl], pp_[:])

        def emit_g_block(gbk):  # gbk 0..7 : m-chunks 4*gbk .. 4*gbk+3
            gp_ = pb.tile([128, 512], f32, tag="pj", name=f"gp{gbk}")
            for jj in range(4):
                m = 4 * gbk + jj
                for c in range(2):
                    _mm(nc, gp_[:, 128 * jj:128 * (jj + 1)],
                        yh_t[c][:, 128 * m:128 * (m + 1)], wg_h[c],
                        start=(c == 0), stop=(c == 1))
            nc.vector.tensor_copy(g_sb[:, 512 * gbk:512 * (gbk + 1)], gp_[:])

        # ---------------- per-half state ----------------
        wy_sb = [mp.tile([128, NL], f16, tag=f"wy{c}", bufs=1, name=f"wy_sb{c}")
                 for c in range(2)]
        H = {}
        sq_sum = {}    # h -> [128,2] f32 sum of wy^2 per channel group
        slin = {}      # h -> [128,1] f32 sum_n y2norm
        mpart = {}     # h -> psum [IC,512] with mean-part in cols 0:2

        def begin_half(h):
            s = {}
            s["n0"] = NH * h
            pool = pa if h == 0 else pb
            tag = "y2a" if h == 0 else "pj"
            s["y2"] = [pool.tile([IC, 512], f32, tag=tag, bufs=2,
                                 name=f"y2p{h}_{j}") for j in range(2)]
            s["dacc"] = [mp.tile([128, NH], DT_ACC, tag=f"dacc{a}", bufs=2,
                                 name=f"dacc{a}_{h}") for a in range(3)]
            s["pend"] = [None, None, None]   # first exp tile per accumulator
            s["dst"] = [False, False, False]
            s["y2q"] = None                  # delayed-y2: exp tile of m-1
            H[h] = s
            s["ft"] = emit_f(h, 0)

        def emit_f(h, m):
            ft = pq.tile([128, 1024], f32, tag="q", name=f"ft{h}_{m}")
            n0 = H[h]["n0"]
            for j in range(2):
                _mm(nc, ft[:, 512 * j:512 * (j + 1)],
                    phi_h[:, 128 * m:128 * (m + 1)],
                    theta_h[:, n0 + 512 * j:n0 + 512 * (j + 1)])
            return ft

        def emit_y2(h, m, expP):
            s = H[h]
            for j in range(2):
                _mm(nc, s["y2"][j][:], g_sb[:, 128 * m:128 * (m + 1)],
                    expP[:, 512 * j:512 * (j + 1)],
                    start=(m == 0), stop=(m == 31))

        def emit_dq(h):
            # d[n] = colsum over m of exp via ones-matmuls on dacc tiles
            s = H[h]
            dqt = pq.tile([128, 1024], f32, tag="q", name=f"dq{h}")
            for j in range(2):
                jsl = slice(512 * j, 512 * (j + 1))
                for a in range(3):
                    rhs = s["dacc"][a][:, jsl]
                    lhs = ones_mb[:] if DACC16 else ones_mf[:].bitcast(f32r)
                    _mm(nc, dqt[0:1, jsl], lhs, rhs,
                        start=(a == 0), stop=(a == 2))
            rinv = mp.tile([1, NH], f32, tag="rinv", bufs=2, name=f"ri{h}")
            nc.vector.reciprocal_approx_fast(rinv[:], dqt[0:1, :])
            s["rinv"] = rinv

        def emit_rb(h):
            # broadcast rinv across partitions: rank-1 f32 matmul + SBUF stage
            s = H[h]
            rbq = pq.tile([128, 1024], f32, tag="q", name=f"rbq{h}")
            for j in range(2):
                jsl = slice(512 * j, 512 * (j + 1))
                _mm(nc, rbq[:, jsl], ones_r[:], s["rinv"][:, jsl])
            rb_sb = mp.tile([128, NH], f32, tag="rb", bufs=2, name=f"rb{h}")
            nc.vector.tensor_copy(rb_sb[:], rbq[:])
            s["rb"] = rb_sb

        def emit_y2norm(h):
            s = H[h]
            y2sb = mp.tile([IC, NH], bf16, tag="y2sb", bufs=2, name=f"y2sb{h}")
            for j in range(2):
                jsl = slice(512 * j, 512 * (j + 1))
                nc.vector.tensor_tensor(y2sb[:, jsl], s["y2"][j][:],
                                        s["rb"][:, jsl], op=ALU.mult)
            s["y2sb"] = y2sb

        def emit_wy(h, c, on_scalar):
            s = H[h]
            n0 = s["n0"]
            for j in range(2):
                jsl = slice(512 * j, 512 * (j + 1))
                wyp = pa.tile([IC, 512], f32, tag="y2a", bufs=2,
                              name=f"wyp{h}_{c}_{j}")
                _mm(nc, wyp[:], wwT_b[:, 128 * c:128 * (c + 1)],
                    s["y2sb"][:, jsl])
                dst = wy_sb[c][:, n0 + 512 * j:n0 + 512 * (j + 1)]
                if on_scalar:
                    nc.scalar.copy(dst, wyp[:])
                else:
                    nc.vector.tensor_copy(dst, wyp[:])

        def emit_sq(h, c, on_scalar):
            s = H[h]
            n0 = s["n0"]
            src = wy_sb[c][:, n0:n0 + NH]
            if h not in sq_sum:
                sq_sum[h] = bp.tile([128, 2], f32, tag=f"sq{h}", name=f"sq{h}")
            acc = sq_sum[h][:, c:c + 1]
            sqt = ep.tile([128, 1024], f16, tag="sqs", bufs=2,
                          name=f"sqt{h}{c}")
            if on_scalar:
                nc.scalar.activation(sqt[:], src, ACTF.Square, accum_out=acc)
            else:
                nc.vector.tensor_tensor(sqt[:], src, src, op=ALU.mult)
                nc.vector.tensor_reduce(acc, sqt[:],
                                        mybir.AxisListType.X, ALU.add)

        def emit_slin(h):
            acc = bp.tile([128, 1], f32, tag=f"slin{h}", name=f"slin{h}")
            nc.vector.tensor_reduce(acc[:], H[h]["y2sb"][:],
                                    mybir.AxisListType.X, ALU.add)
            slin[h] = acc

        def emit_mpart(h):
            # mean-part = wwT.T @ slin_h  (exact f32 1-col matmuls)
            mps = pa.tile([IC, 512], f32, tag="y2a", bufs=2, name=f"mps{h}")
            for c in range(2):
                _mm(nc, mps[:, c:c + 1], w2[:, 128 * c:128 * (c + 1)],
                    slin[h][:])
            mpart[h] = mps

        packed = bp.tile([128, 8], f32, tag="packed")

        def emit_pack(h):
            nc.vector.tensor_copy(packed[:, 4 * h:4 * h + 2],
                                  mpart[h][:, 0:2])
            nc.vector.tensor_copy(packed[:, 4 * h + 2:4 * h + 4],
                                  sq_sum[h][:])

        def emit_barrier_round():
            if not RDMA:
                return
            for delta in range(1, 8):
                rdests = [None] * 8
                rdests[delta] = (0, delta)
                nc.gpsimd.remote_sem_update_broadcast(
                    bar_sem, loc_sem, rdests=rdests)
            nc.gpsimd.trigger_dma(count=None)

        # ---------------- main loop ----------------
        def emit_iter(h, m, slot=None):
            s = H[h]
            ft_cur = s["ft"]
            if m < 31:
                s["ft"] = emit_f(h, m + 1)
            if slot is not None:
                slot()
            if s["y2q"] is not None:
                emit_y2(h, m - 1, s["y2q"])
            expP = ep.tile([128, 1024], bf16, tag="exp", name=f"ex{h}_{m}")
            nc.scalar.activation(expP[:], ft_cur[:], ACTF.Exp)
            s["y2q"] = expP
            # d-adds: 3-of-4 on DVE (bf16 2x), 1-of-4 on Pool (SBUF-only);
            # accumulators initialized by summing their first two exp tiles
            a = 2 if (m & 3) == 3 else (m & 1)
            eng = nc.gpsimd if a == 2 else nc.vector
            acc = s["dacc"][a]
            if not s["dst"][a]:
                if s["pend"][a] is None:
                    s["pend"][a] = expP
                else:
                    eng.tensor_tensor(acc[:], s["pend"][a][:], expP[:],
                                      op=ALU.add)
                    s["pend"][a] = None
                    s["dst"][a] = True
            else:
                eng.tensor_tensor(acc[:], acc[:], expP[:], op=ALU.add)

        # --- pre-loop projections (y chunk 0 dependent) ---
        emit_theta_block(0)
        emit_theta_block(1)
        emit_phi_block(0)
        emit_g_block(0)
        emit_phi_block(1)
        emit_g_block(1)

        h0_slots = {
            2: lambda: emit_phi_block(2),
            3: lambda: emit_g_block(2),
            4: lambda: emit_y_dma(2),
            6: lambda: emit_phi_block(3),
            7: lambda: emit_g_block(3),
            8: lambda: emit_theta_block(2),
            9: lambda: emit_theta_block(3),
            10: lambda: emit_y_dma(3),
            12: lambda: emit_phi_block(4),
            13: lambda: emit_g_block(4),
            16: lambda: emit_phi_block(5),
            17: lambda: emit_g_block(5),
            20: lambda: emit_phi_block(6),
            21: lambda: emit_g_block(6),
            24: lambda: emit_phi_block(7),
            25: lambda: emit_g_block(7),
        }
        h1_slots = {
            1: lambda: emit_dq(0),
            2: lambda: emit_rb(0),
            3: lambda: emit_y2norm(0),
            4: lambda: emit_wy(0, 0, on_scalar=False),
            5: lambda: emit_wy(0, 1, on_scalar=False),
            7: lambda: emit_sq(0, 0, on_scalar=False),
            9: lambda: emit_sq(0, 1, on_scalar=False),
            11: lambda: emit_slin(0),
            12: lambda: emit_mpart(0),
            13: lambda: emit_pack(0),
            16: emit_barrier_round,
        }

        with nc.allow_low_precision("bf16 softmax denominator accumulate"):
            begin_half(0)
            for m in range(32):
                emit_iter(0, m, h0_slots.get(m))
            emit_y2(0, 31, H[0]["y2q"])
            begin_half(1)
            for m in range(32):
                emit_iter(1, m, h1_slots.get(m))
            emit_y2(1, 31, H[1]["y2q"])

            # ---------------- tail: half 1 norm + wy + stats ----------
            emit_dq(1)
            emit_rb(1)
            emit_y2norm(1)
            emit_wy(1, 0, on_scalar=True)
            emit_wy(1, 1, on_scalar=True)
            emit_slin(1)
            emit_sq(1, 0, on_scalar=True)
            emit_sq(1, 1, on_scalar=True)
        emit_mpart(1)
        emit_pack(1)

        # ---------------- stats exchange ----------------
        gstats = bp.tile([128, 4], f32, tag="gstats")
        if RDMA:
            # stats sends: preps emitted now, data dep lands on the trigger
            for delta in range(1, 8):
                rdests = [None] * 8
                rdests[delta] = (0, delta)
                nc.gpsimd.remote_dma_broadcast(
                    rdrecv[:, 8 * (delta - 1):8 * delta], packed[:],
                    remote_sem=st_sem, local_sem=loc_sem, rdests=rdests)
            nc.gpsimd.wait_ge(bar_sem, 14)    # align cores
            nc.gpsimd.trigger_dma(count=None)
            nc.vector.wait_ge(st_sem, 14)     # all 7 peers landed
            tot = bp.tile([128, 8], f32, tag="tot")
            nc.vector.tensor_tensor(tot[:], packed[:], rdrecv[:, 0:8],
                                    op=ALU.add)
            for k in range(1, 7):
                nc.vector.tensor_tensor(tot[:], tot[:],
                                        rdrecv[:, 8 * k:8 * (k + 1)],
                                        op=ALU.add)
            for c in range(4):
                nc.vector.tensor_tensor(gstats[:, c:c + 1], tot[:, c:c + 1],
                                        tot[:, 4 + c:5 + c], op=ALU.add)
        else:
            ar_in = dr.tile([128, 8], f32, name="ar_in")
            ar_out = dr.tile([128, 8], f32, name="ar_out")
            nc.sync.dma_start(ar_in[:], packed[:])
            nc.gpsimd.collective_compute(
                "AllReduce", ALU.add,
                replica_groups=[list(range(N_CORES))],
                ins=[ar_in.opt()], outs=[ar_out.opt()])
            gsb = bp.tile([128, 8], f32, tag="gsb")
            nc.sync.dma_start(gsb[:], ar_out[:])
            for c in range(4):
                nc.vector.tensor_tensor(gstats[:, c:c + 1], gsb[:, c:c + 1],
                                        gsb[:, 4 + c:5 + c], op=ALU.add)

        # ---------------- BN math + apply + residual ----------------
        # gstats cols: [meansum_c0, meansum_c1, sqsum_c0, sqsum_c1]
        for c in range(2):
            mean = bp.tile([128, 1], f32, tag=f"mean{c}")
            nc.vector.tensor_scalar(mean[:], gstats[:, c:c + 1],
                                    1.0 / CNT, None, ALU.mult)
            msq = bp.tile([128, 1], f32, tag=f"msq{c}")
            nc.vector.tensor_scalar(msq[:], gstats[:, 2 + c:3 + c],
                                    1.0 / CNT, None, ALU.mult)
            m2 = bp.tile([128, 1], f32, tag=f"m2{c}")
            nc.vector.tensor_tensor(m2[:], mean[:], mean[:], op=ALU.mult)
            var = bp.tile([128, 1], f32, tag=f"var{c}")
            nc.vector.tensor_tensor(var[:], msq[:], m2[:], op=ALU.subtract)
            varep = bp.tile([128, 1], f32, tag=f"varep{c}")
            nc.vector.tensor_scalar(varep[:], var[:], float(EPS), None, ALU.add)
            sd = bp.tile([128, 1], f32, tag=f"sd{c}")
            nc.scalar.activation(sd[:], varep[:], ACTF.Sqrt)
            rstd = bp.tile([128, 1], f32, tag=f"rstd{c}")
            nc.vector.reciprocal(rstd[:], sd[:])
            scale = bp.tile([128, 1], f32, tag=f"scale{c}")
            nc.vector.tensor_tensor(scale[:], gamma_t[c], rstd[:], op=ALU.mult)
            msc = bp.tile([128, 1], f32, tag=f"msc{c}")
            nc.vector.tensor_tensor(msc[:], mean[:], scale[:], op=ALU.mult)
            shift = bp.tile([128, 1], f32, tag=f"shift{c}")
            nc.vector.tensor_tensor(shift[:], beta_t[c], msc[:], op=ALU.subtract)

            out_t = mp.tile([128, NL], f32, tag=f"out{c}", bufs=1,
                            name=f"out{c}")
            for k in range(2):
                sl = slice(1024 * k, 1024 * (k + 1))
                nc.vector.affine_then_add(out_t[:, sl], wy_sb[c][:, sl],
                                          xl_t[c][:, sl], scale[:], shift[:])
                nc.sync.dma_start(out_d[128 * c:128 * (c + 1), sl],
                                  out_t[:, sl])


_NC_CACHE = None


def _get_nc():
    global _NC_CACHE
    if _NC_CACHE is None:
        _NC_CACHE = _build()
    return _NC_CACHE


def shard_inputs(inputs):
    x = np.ascontiguousarray(inputs["x"], dtype=np.float32).reshape(B, C, N)
    y = np.ascontiguousarray(inputs["y"], dtype=np.float32).reshape(B, C, N)
    dxwT = np.asarray(inputs["dx_w"]).T.astype(np.float32)
    dywT = np.asarray(inputs["dy_w"]).T.astype(np.float32)
    gwT = np.asarray(inputs["g_w"]).T.astype(np.float32)
    wwT = np.asarray(inputs["w_w"]).T.astype(np.float32)
    dxb = np.asarray(inputs["dx_b"], dtype=np.float32).reshape(IC, 1)
    gamma = np.asarray(inputs["bn_gamma"], dtype=np.float32).reshape(C, 1)
    beta = np.asarray(inputs["bn_beta"], dtype=np.float32).reshape(C, 1)
    # pack all small weights into two tensors (3 DMAs instead of 12)
    wpk = np.ascontiguousarray(
        np.concatenate([dxwT, dywT, gwT, gamma, beta], axis=1))   # [256, 386]
    wpk2 = np.ascontiguousarray(
        np.concatenate([wwT, dxb], axis=1))                        # [128, 257]

    in_maps = []
    for core in range(N_CORES):
        b, h = divmod(core, 2)
        in_maps.append({
            "xl": np.ascontiguousarray(x[b][:, h * NL:(h + 1) * NL]),
            "yl": y[b],
            "wpk": wpk, "wpk2": wpk2,
        })
    return in_maps


def run(inputs, **kw):
    """Run on hardware; returns (full_output, BassKernelResults)."""
    nc = _get_nc()
    in_maps = shard_inputs(inputs)
    r = run_bass_kernel_spmd(nc, in_maps, core_ids=list(range(N_CORES)), **kw)
    out = np.empty((B, C, N), np.float32)
    for core in range(N_CORES):
        b, h = divmod(core, 2)
        out[b][:, h * NL:(h + 1) * NL] = r.results[core]["out"]
    return out.reshape(B, C, HW, HW), r


def kernel(**inputs):
    out, _ = run(inputs)
    return out


# revision 25
# speedup vs baseline: 1.0858x; 1.0858x over previous
"""Trainium2 Bass kernel for nn_AttentionBlock (B=4, C=256, H=W=64, IC=128).

Sharding: 8 cores = 4 batches x 2 row-halves of the N=4096 attention dim.
Each core computes its 2048 rows of the attention output, the final 1x1 conv
(wy), and partial BatchNorm statistics; a peer-to-peer SBUF exchange (or an
AllReduce fallback) combines the BN stats; each core then applies BN +
residual and writes its output slice.

Algebraic simplifications vs the reference (all exact):
  - g_b and w_b only add a per-channel constant to wy, which BatchNorm's
    mean subtraction cancels -> dropped.
  - dy_b (phi bias) only adds row-constant terms to the attention logits,
    which softmax cancels -> dropped. Only dx_b (theta bias) is applied.
  - softmax computed without max-subtraction: logits are bounded
    (|f| < ~70 for randn inputs), within bf16 exp range.
  - BN linear sums computed as wwT.T @ sum_n(y2norm) instead of summing wy.

v4 structure (209us v2 baseline -> this):
  - Softmax denominator accumulated in bf16 (2x DVE mode) with three
    accumulators: 3-of-4 adds on DVE, 1-of-4 on Pool; accumulators
    initialized by adding two exp tiles (no init copies); final
    column-reduce via 6 bf16 ones-matmuls on the PE.
  - Scalar engine runs ONLY exp in the main loop (64 x ~1.05us = floor).
  - y2 matmuls are emitted one iteration LATE (delayed-y2): every PE
    instruction's dependencies are satisfied a full iteration ahead, so
    the PE stream has no semaphore gaps and holds its full 2.4GHz p-state.
  - Projections interleaved in fine-grained blocks into the h0 loop.
  - Weights DMA'd first (200KB) so the first theta matmul isn't gated on
    the 6MB x/y stream.
  - BN stats exchanged via direct SBUF->SBUF remote DMA between the 8
    cores (XOR-rotation addressing, SPMD-safe), preceded by a sem-only
    barrier round for alignment: ~4us exposed vs ~30us for the DRAM
    collective. K_RDMA=0 falls back to a single AllReduce.
"""

import os
import sys
import numpy as np

if "/opt/trn_rl_repo" not in sys.path:
    sys.path.insert(0, "/opt/trn_rl_repo")

import concourse.bass as bass
import concourse.bacc as bacc
import concourse.mybir as mybir
import concourse.tile as tile
from concourse.bass_utils import run_bass_kernel_spmd

N_CORES = 8
B, C, HW = 4, 256, 64
N = HW * HW          # 4096 spatial positions per batch
IC = 128             # inter channels
NL = N // 2          # 2048 rows per core
NH = NL // 2         # 1024 rows per attention half
EPS = 1e-5
CNT = float(B * N)   # BatchNorm count per channel

f32 = mybir.dt.float32
f32r = mybir.dt.float32r
bf16 = mybir.dt.bfloat16
f16 = mybir.dt.float16
ALU = mybir.AluOpType
ACTF = mybir.ActivationFunctionType

DACC16 = os.environ.get("K_DACC16", "1") == "1"   # bf16 d-accumulators
RDMA = os.environ.get("K_RDMA", "0") == "1"       # p2p stats exchange


def _mm(nc, out, lhsT, rhs, start=True, stop=True):
    return nc.tensor.matmul(out, lhsT, rhs, start=start, stop=stop)


def _build():
    nc = bacc.Bacc("TRN2", target_bir_lowering=False, debug=False,
                   num_devices=N_CORES)

    xl_d = nc.dram_tensor("xl", [C, NL], f32, kind="ExternalInput").ap()
    yl_d = nc.dram_tensor("yl", [C, N], f32, kind="ExternalInput").ap()
    wpk_d = nc.dram_tensor("wpk", [C, 386], f32, kind="ExternalInput").ap()
    wpk2_d = nc.dram_tensor("wpk2", [IC, 257], f32, kind="ExternalInput").ap()
    out_d = nc.dram_tensor("out", [C, NL], f32, kind="ExternalOutput").ap()

    with tile.TileContext(nc) as tc:
        _emit(nc, tc, xl_d, yl_d, wpk_d, wpk2_d, out_d)
    nc.compile()
    return nc


def _emit(nc, tc, xl_d, yl_d, wpk_d, wpk2_d, out_d):
    DT_ACC = bf16 if DACC16 else f32r
    with (
        tc.tile_pool(name="sb_w", bufs=1) as wp,        # weights + tiny tiles
        tc.tile_pool(name="sb_x", bufs=1) as xp,        # x / y staging
        tc.tile_pool(name="sb_a", bufs=1) as ap_,       # theta/phi/g activations
        tc.tile_pool(name="sb_e", bufs=8) as ep,        # exp tiles
        tc.tile_pool(name="sb_m", bufs=2) as mp,        # misc per-half tiles
        tc.tile_pool(name="sb_bn", bufs=1) as bp,       # bn tiny tiles
        tc.tile_pool(name="ps_q", bufs=2, space="PSUM") as pq,    # ft/dq/rb
        tc.tile_pool(name="ps_a", bufs=2, space="PSUM") as pa,    # y2 h0 / wyp
        tc.tile_pool(name="ps_b", bufs=2, space="PSUM") as pb,    # proj / y2 h1
        tc.tile_pool(name="dram", bufs=1, space="DRAM") as dr,
    ):
        # ---------------- DMAs: weights first (small), then x t0, y0 -------
        w1 = [wp.tile([128, 386], f32, tag=f"w1_{i}", name=f"w1_{i}")
              for i in range(2)]
        w2 = wp.tile([IC, 257], f32, tag="w2")
        for i in range(2):
            nc.sync.dma_start(w1[i][:], wpk_d[128 * i:128 * (i + 1), :])
        nc.sync.dma_start(w2[:], wpk2_d[:])

        xl_t = [xp.tile([128, NL], f32, tag=f"xl{c}", bufs=1, name=f"xl{c}")
                for c in range(2)]
        for c in range(2):
            nc.sync.dma_start(xl_t[c][:, 0:NH], xl_d[128 * c:128 * (c + 1), 0:NH])

        yh_t = [xp.tile([128, N], f16, tag=f"yh{c}", bufs=1, name=f"yh{c}")
                for c in range(2)]

        def emit_y_dma(t):
            sl = slice(1024 * t, 1024 * (t + 1))
            for c in range(2):
                nc.gpsimd.dma_start(yh_t[c][:, sl],
                                    yl_d[128 * c:128 * (c + 1), sl])

        emit_y_dma(0)
        for c in range(2):
            nc.sync.dma_start(xl_t[c][:, NH:NL], xl_d[128 * c:128 * (c + 1), NH:NL])

        # ---------------- weight casts (DVE) ----------------
        # only what the first theta/phi/g blocks need; the rest is deferred
        # into loop slots so the pre-loop DVE chain stays short
        wh1 = [wp.tile([128, 384], f16, tag=f"wh1_{i}", name=f"wh1_{i}")
               for i in range(2)]
        for i in range(2):
            nc.vector.tensor_copy(wh1[i][:], w1[i][:, 0:384])
        wdx_h = [wh1[i][:, 0:128] for i in range(2)]
        wdy_h = [wh1[i][:, 128:256] for i in range(2)]
        wg_h = [wh1[i][:, 256:384] for i in range(2)]
        gamma_t = [w1[i][:, 384:385] for i in range(2)]
        beta_t = [w1[i][:, 385:386] for i in range(2)]

        xh_t = [xp.tile([128, NL], f16, tag=f"xh{c}", bufs=1, name=f"xh{c}")
                for c in range(2)]
        for c in range(2):
            nc.vector.tensor_copy(xh_t[c][:, 0:NH], xl_t[c][:, 0:NH])
        dxb_t = wp.tile([IC, 1], f32, tag="dxb")
        nc.vector.tensor_copy(dxb_t[:], w2[:, 256:257])

        wwT_b = wp.tile([IC, C], bf16, tag="wwT_b")
        ones_mb = wp.tile([128, 1], bf16, tag="ones_mb")  # d-reduce stationary
        ones_mf = wp.tile([128, 1], f32, tag="ones_mf")
        ones_rb = wp.tile([1, 128], bf16, tag="ones_rb")  # rinv bcast stationary

        def emit_deferred_w():
            nc.vector.tensor_copy(wwT_b[:], w2[:, 0:256])
            nc.vector.memset(ones_mb[:], 1.0)
            nc.vector.memset(ones_mf[:], 1.0)
            nc.vector.memset(ones_rb[:], 1.0)

        if RDMA:
            # stats recv buffer: written ONLY by remote cores; memset early
            # so the allocator sees a writer (peers write >80us later)
            rdrecv = wp.tile([128, 56], f32, tag="rdrecv")   # 7 slots x 8
            nc.vector.memset(rdrecv[:], 0.0)
            bar_sem = nc.alloc_semaphore("bn_bar")
            st_sem = nc.alloc_semaphore("bn_st")
            loc_sem = nc.alloc_semaphore("bn_loc")

        def emit_xh_t1(c):
            nc.vector.tensor_copy(xh_t[c][:, NH:NL], xl_t[c][:, NH:NL])

        # ---------------- projection targets ----------------
        theta_h = ap_.tile([IC, NL], f16, tag="theta")
        phi_h = ap_.tile([IC, N], f16, tag="phi")
        g_sb = ap_.tile([128, N], bf16, tag="g")   # 32 chunks [m128, ic128]

        def emit_theta_block(tb):  # tb 0..3 : theta n-cols 512*tb
            sl = slice(512 * tb, 512 * (tb + 1))
            tp_ = pb.tile([128, 512], f32, tag="pj", name=f"thp{tb}")
            for c in range(2):
                _mm(nc, tp_[:], wdx_h[c], xh_t[c][:, sl],
                    start=(c == 0), stop=(c == 1))
            nc.vector.tensor_scalar(theta_h[:, sl], tp_[:], dxb_t[:],
                                    None, ALU.add)

        def emit_phi_block(pbk):  # pbk 0..7 : phi m-cols 512*pbk
            sl = slice(512 * pbk, 512 * (pbk + 1))
            pp_ = pb.tile([128, 512], f32, tag="pj", name=f"php{pbk}")
            for c in range(2):
                _mm(nc, pp_[:], wdy_h[c], yh_t[c][:, sl],
                    start=(c == 0), stop=(c == 1))
            nc.vector.tensor_copy(phi_h[:, sl], pp_[:])

        def emit_g_block(gbk):  # gbk 0..7 : m-chunks 4*gbk .. 4*gbk+3
            gp_ = pb.tile([128, 512], f32, tag="pj", name=f"gp{gbk}")
            for jj in range(4):
                m = 4 * gbk + jj
                for c in range(2):
                    _mm(nc, gp_[:, 128 * jj:128 * (jj + 1)],
                        yh_t[c][:, 128 * m:128 * (m + 1)], wg_h[c],
                        start=(c == 0), stop=(c == 1))
            nc.vector.tensor_copy(g_sb[:, 512 * gbk:512 * (gbk + 1)], gp_[:])

        # ---------------- per-half state ----------------
        wy_sb = [mp.tile([128, NL], f16, tag=f"wy{c}", bufs=1, name=f"wy_sb{c}")
                 for c in range(2)]
        H = {}
        sq_sum = {}    # h -> [128,2] f32 sum of wy^2 per channel group
        slin = {}      # h -> [128,1] f32 sum_n y2norm
        mpart = {}     # h -> psum [IC,512] with mean-part in cols 0:2

        def begin_half(h):
            s = {}
            s["n0"] = NH * h
            pool = pa if h == 0 else pb
            tag = "y2a" if h == 0 else "pj"
            s["y2"] = [pool.tile([IC, 512], f32, tag=tag, bufs=2,
                                 name=f"y2p{h}_{j}") for j in range(2)]
            s["dacc"] = [mp.tile([128, NH], DT_ACC, tag=f"dacc{a}", bufs=2,
                                 name=f"dacc{a}_{h}") for a in range(3)]
            s["pend"] = [None, None, None]   # first exp tile per accumulator
            s["dst"] = [False, False, False]
            s["y2q"] = None                  # delayed-y2: exp tile of m-1
            H[h] = s
            s["ft"] = emit_f(h, 0)

        def emit_f(h, m):
            ft = pq.tile([128, 1024], f32, tag="q", name=f"ft{h}_{m}")
            n0 = H[h]["n0"]
            for j in range(2):
                _mm(nc, ft[:, 512 * j:512 * (j + 1)],
                    phi_h[:, 128 * m:128 * (m + 1)],
                    theta_h[:, n0 + 512 * j:n0 + 512 * (j + 1)])
            return ft

        def emit_y2(h, m, expP):
            s = H[h]
            for j in range(2):
                _mm(nc, s["y2"][j][:], g_sb[:, 128 * m:128 * (m + 1)],
                    expP[:, 512 * j:512 * (j + 1)],
                    start=(m == 0), stop=(m == 31))

        def emit_dq(h):
            # d[n] = colsum over m of exp via ones-matmuls on dacc tiles
            s = H[h]
            dqt = pq.tile([128, 1024], f32, tag="q", name=f"dq{h}")
            for j in range(2):
                jsl = slice(512 * j, 512 * (j + 1))
                for a in range(3):
                    rhs = s["dacc"][a][:, jsl]
                    lhs = ones_mb[:] if DACC16 else ones_mf[:].bitcast(f32r)
                    _mm(nc, dqt[0:1, jsl], lhs, rhs,
                        start=(a == 0), stop=(a == 2))
            # rinv in bf16 (0.4% quantization, fine for the 2e-2 budget)
            # so the partition-broadcast matmul runs at 1 cycle/row
            rinv = mp.tile([1, NH], bf16, tag="rinv", bufs=2, name=f"ri{h}")
            nc.vector.reciprocal(rinv[:], dqt[0:1, :])
            s["rinv"] = rinv

        def emit_rb(h):
            # broadcast rinv across partitions: rank-1 bf16 matmul + SBUF stage
            s = H[h]
            rbq = pq.tile([128, 1024], f32, tag="q", name=f"rbq{h}")
            for j in range(2):
                jsl = slice(512 * j, 512 * (j + 1))
                _mm(nc, rbq[:, jsl], ones_rb[:], s["rinv"][:, jsl])
            rb_sb = mp.tile([128, NH], f32, tag="rb", bufs=2, name=f"rb{h}")
            nc.vector.tensor_copy(rb_sb[:], rbq[:])
            s["rb"] = rb_sb

        def emit_y2norm(h):
            s = H[h]
            y2sb = mp.tile([IC, NH], bf16, tag="y2sb", bufs=2, name=f"y2sb{h}")
            for j in range(2):
                jsl = slice(512 * j, 512 * (j + 1))
                nc.vector.tensor_tensor(y2sb[:, jsl], s["y2"][j][:],
                                        s["rb"][:, jsl], op=ALU.mult)
            s["y2sb"] = y2sb

        def emit_wy(h, c, on_scalar):
            s = H[h]
            n0 = s["n0"]
            for j in range(2):
                jsl = slice(512 * j, 512 * (j + 1))
                wyp = pa.tile([IC, 512], f32, tag="y2a", bufs=2,
                              name=f"wyp{h}_{c}_{j}")
                _mm(nc, wyp[:], wwT_b[:, 128 * c:128 * (c + 1)],
                    s["y2sb"][:, jsl])
                dst = wy_sb[c][:, n0 + 512 * j:n0 + 512 * (j + 1)]
                if on_scalar:
                    nc.scalar.copy(dst, wyp[:])
                else:
                    nc.vector.tensor_copy(dst, wyp[:])

        def emit_sq(h, c, on_scalar):
            s = H[h]
            n0 = s["n0"]
            src = wy_sb[c][:, n0:n0 + NH]
            if h not in sq_sum:
                sq_sum[h] = bp.tile([128, 2], f32, tag=f"sq{h}", name=f"sq{h}")
            acc = sq_sum[h][:, c:c + 1]
            sqt = ep.tile([128, 1024], f16, tag="sqs", bufs=2,
                          name=f"sqt{h}{c}")
            if on_scalar:
                nc.scalar.activation(sqt[:], src, ACTF.Square, accum_out=acc)
            else:
                nc.vector.tensor_tensor(sqt[:], src, src, op=ALU.mult)
                nc.vector.tensor_reduce(acc, sqt[:],
                                        mybir.AxisListType.X, ALU.add)

        def emit_slin(h):
            acc = bp.tile([128, 1], f32, tag=f"slin{h}", name=f"slin{h}")
            nc.vector.tensor_reduce(acc[:], H[h]["y2sb"][:],
                                    mybir.AxisListType.X, ALU.add)
            slin[h] = acc

        def emit_mpart(h):
            # mean-part = wwT.T @ slin_h  (exact f32 1-col matmuls)
            mps = pa.tile([IC, 512], f32, tag="y2a", bufs=2, name=f"mps{h}")
            for c in range(2):
                _mm(nc, mps[:, c:c + 1], w2[:, 128 * c:128 * (c + 1)],
                    slin[h][:])
            mpart[h] = mps

        packed = [bp.tile([128, 4], f32, tag=f"packed{h}", name=f"packed{h}")
                  for h in range(2)]
        ar_io = {}

        def emit_pack(h):
            if h == 1 and not RDMA:
                # gate h1's pack on AR0 completion: (gsb0 * 0) + mpart —
                # aligns all cores' AR1 arrival on AR0's global finish
                nc.vector.scalar_tensor_tensor(
                    packed[1][:, 0:2], ar_io[0][2][:, 0:2], 0.0,
                    mpart[1][:, 0:2], op0=ALU.mult, op1=ALU.add)
            else:
                nc.vector.tensor_copy(packed[h][:, 0:2], mpart[h][:, 0:2])
            nc.vector.tensor_copy(packed[h][:, 2:4], sq_sum[h][:])

        def emit_exchange(h):
            if RDMA:
                return
            ar_in = dr.tile([128, 4], f32, name=f"ar_in{h}")
            ar_out = dr.tile([128, 4], f32, name=f"ar_out{h}")
            nc.sync.dma_start(ar_in[:], packed[h][:])
            nc.gpsimd.collective_compute(
                "AllReduce", ALU.add,
                replica_groups=[list(range(N_CORES))],
                ins=[ar_in.opt()], outs=[ar_out.opt()])
            gsb = bp.tile([128, 4], f32, tag=f"gsb{h}", name=f"gsb{h}")
            nc.sync.dma_start(gsb[:], ar_out[:])
            ar_io[h] = (ar_in, ar_out, gsb)

        def emit_barrier_round():
            if not RDMA:
                return
            for delta in range(1, 8):
                rdests = [None] * 8
                rdests[delta] = (0, delta)
                nc.gpsimd.remote_sem_update_broadcast(
                    bar_sem, loc_sem, rdests=rdests)
            nc.gpsimd.trigger_dma(count=None)

        # ---------------- main loop ----------------
        def emit_iter(h, m, slot=None):
            s = H[h]
            ft_cur = s["ft"]
            if m < 31:
                s["ft"] = emit_f(h, m + 1)
            if slot is not None:
                slot()
            if s["y2q"] is not None:
                emit_y2(h, m - 1, s["y2q"])
            expP = ep.tile([128, 1024], bf16, tag="exp", name=f"ex{h}_{m}")
            nc.scalar.activation(expP[:], ft_cur[:], ACTF.Exp)
            s["y2q"] = expP
            # d-adds: 3-of-4 on DVE (bf16 2x), 1-of-4 on Pool (SBUF-only);
            # accumulators initialized by summing their first two exp tiles.
            # Pool's adds are slow (~2.1us) so keep it off the last iters:
            # dq() must not wait on a straggling Pool add.
            a = 2 if (m & 3) == 3 and m < 28 else (m & 1)
            eng = nc.gpsimd if a == 2 else nc.vector
            acc = s["dacc"][a]
            if not s["dst"][a]:
                if s["pend"][a] is None:
                    s["pend"][a] = expP
                else:
                    eng.tensor_tensor(acc[:], s["pend"][a][:], expP[:],
                                      op=ALU.add)
                    s["pend"][a] = None
                    s["dst"][a] = True
            else:
                eng.tensor_tensor(acc[:], acc[:], expP[:], op=ALU.add)

        # --- pre-loop projections (y chunk 0 dependent) ---
        emit_theta_block(0)
        emit_theta_block(1)
        emit_phi_block(0)
        emit_g_block(0)
        emit_phi_block(1)
        emit_g_block(1)

        h0_slots = {
            0: lambda: emit_y_dma(1),
            2: lambda: emit_phi_block(2),
            3: lambda: emit_g_block(2),
            4: lambda: emit_y_dma(2),
            5: lambda: emit_xh_t1(0),
            6: lambda: emit_xh_t1(1),
            7: lambda: emit_phi_block(3),
            8: lambda: emit_g_block(3),
            9: lambda: emit_theta_block(2),
            10: lambda: emit_theta_block(3),
            11: emit_deferred_w,
            12: lambda: emit_y_dma(3),
            13: lambda: emit_phi_block(4),
            14: lambda: emit_g_block(4),
            16: lambda: emit_phi_block(5),
            17: lambda: emit_g_block(5),
            20: lambda: emit_phi_block(6),
            21: lambda: emit_g_block(6),
            24: lambda: emit_phi_block(7),
            25: lambda: emit_g_block(7),
        }
        h1_slots = {
            3: lambda: emit_dq(0),
            4: lambda: emit_rb(0),
            5: lambda: emit_y2norm(0),
            7: lambda: emit_wy(0, 0, on_scalar=False),
            9: lambda: emit_wy(0, 1, on_scalar=False),
            11: lambda: emit_sq(0, 0, on_scalar=False),
            13: lambda: emit_sq(0, 1, on_scalar=False),
            14: lambda: emit_slin(0),
            15: lambda: emit_mpart(0),
            16: lambda: (emit_pack(0), emit_exchange(0)),
            18: emit_barrier_round,
        }

        with nc.allow_low_precision("bf16 softmax denominator accumulate"):
            begin_half(0)
            for m in range(32):
                emit_iter(0, m, h0_slots.get(m))
            emit_y2(0, 31, H[0]["y2q"])
            begin_half(1)
            for m in range(32):
                emit_iter(1, m, h1_slots.get(m))
            emit_y2(1, 31, H[1]["y2q"])

            # ---------------- tail: half 1 norm + wy + stats ----------
            emit_dq(1)
            emit_rb(1)
            emit_y2norm(1)
            emit_slin(1)
            emit_wy(1, 0, on_scalar=True)
            emit_wy(1, 1, on_scalar=True)
            emit_sq(1, 0, on_scalar=True)
            emit_sq(1, 1, on_scalar=True)
        emit_mpart(1)
        emit_pack(1)

        # ---------------- stats exchange (h1 round) ----------------
        gstats = bp.tile([128, 4], f32, tag="gstats")
        if RDMA:
            # stats sends: preps emitted now, data dep lands on the trigger
            pk = bp.tile([128, 8], f32, tag="pk8")
            nc.vector.tensor_copy(pk[:, 0:4], packed[0][:])
            nc.vector.tensor_copy(pk[:, 4:8], packed[1][:])
            for delta in range(1, 8):
                rdests = [None] * 8
                rdests[delta] = (0, delta)
                nc.gpsimd.remote_dma_broadcast(
                    rdrecv[:, 8 * (delta - 1):8 * delta], pk[:],
                    remote_sem=st_sem, local_sem=loc_sem, rdests=rdests)
            nc.gpsimd.wait_ge(bar_sem, 14)    # align cores
            nc.gpsimd.trigger_dma(count=None)
            nc.vector.wait_ge(st_sem, 14)     # all 7 peers landed
            tot = bp.tile([128, 8], f32, tag="tot")
            nc.vector.tensor_tensor(tot[:], pk[:], rdrecv[:, 0:8],
                                    op=ALU.add)
            for k in range(1, 7):
                nc.vector.tensor_tensor(tot[:], tot[:],
                                        rdrecv[:, 8 * k:8 * (k + 1)],
                                        op=ALU.add)
            for c in range(4):
                nc.vector.tensor_tensor(gstats[:, c:c + 1], tot[:, c:c + 1],
                                        tot[:, 4 + c:5 + c], op=ALU.add)
        else:
            emit_exchange(1)
            nc.vector.tensor_tensor(gstats[:], ar_io[0][2][:], ar_io[1][2][:],
                                    op=ALU.add)

        # ---------------- BN math + apply + residual ----------------
        # gstats cols: [meansum_c0, meansum_c1, sqsum_c0, sqsum_c1]
        for c in range(2):
            mean = bp.tile([128, 1], f32, tag=f"mean{c}")
            nc.vector.tensor_scalar(mean[:], gstats[:, c:c + 1],
                                    1.0 / CNT, None, ALU.mult)
            msq = bp.tile([128, 1], f32, tag=f"msq{c}")
            nc.vector.tensor_scalar(msq[:], gstats[:, 2 + c:3 + c],
                                    1.0 / CNT, None, ALU.mult)
            m2 = bp.tile([128, 1], f32, tag=f"m2{c}")
            nc.vector.tensor_tensor(m2[:], mean[:], mean[:], op=ALU.mult)
            var = bp.tile([128, 1], f32, tag=f"var{c}")
            nc.vector.tensor_tensor(var[:], msq[:], m2[:], op=ALU.subtract)
            varep = bp.tile([128, 1], f32, tag=f"varep{c}")
            nc.vector.tensor_scalar(varep[:], var[:], float(EPS), None, ALU.add)
            sd = bp.tile([128, 1], f32, tag=f"sd{c}")
            nc.scalar.activation(sd[:], varep[:], ACTF.Sqrt)
            rstd = bp.tile([128, 1], f32, tag=f"rstd{c}")
            nc.vector.reciprocal(rstd[:], sd[:])
            scale = bp.tile([128, 1], f32, tag=f"scale{c}")
            nc.vector.tensor_tensor(scale[:], gamma_t[c], rstd[:], op=ALU.mult)
            msc = bp.tile([128, 1], f32, tag=f"msc{c}")
            nc.vector.tensor_tensor(msc[:], mean[:], scale[:], op=ALU.mult)
            shift = bp.tile([128, 1], f32, tag=f"shift{c}")
            nc.vector.tensor_tensor(shift[:], beta_t[c], msc[:], op=ALU.subtract)

            out_t = mp.tile([128, NL], f32, tag=f"out{c}", bufs=1,
                            name=f"out{c}")
            for k in range(2):
                sl = slice(1024 * k, 1024 * (k + 1))
                nc.vector.affine_then_add(out_t[:, sl], wy_sb[c][:, sl],
                                          xl_t[c][:, sl], scale[:], shift[:])
                nc.sync.dma_start(out_d[128 * c:128 * (c + 1), sl],
                                  out_t[:, sl])


_NC_CACHE = None


def _get_nc():
    global _NC_CACHE
    if _NC_CACHE is None:
        _NC_CACHE = _build()
    return _NC_CACHE


def shard_inputs(inputs):
    x = np.ascontiguousarray(inputs["x"], dtype=np.float32).reshape(B, C, N)
    y = np.ascontiguousarray(inputs["y"], dtype=np.float32).reshape(B, C, N)
    dxwT = np.asarray(inputs["dx_w"]).T.astype(np.float32)
    dywT = np.asarray(inputs["dy_w"]).T.astype(np.float32)
    gwT = np.asarray(inputs["g_w"]).T.astype(np.float32)
    wwT = np.asarray(inputs["w_w"]).T.astype(np.float32)
    dxb = np.asarray(inputs["dx_b"], dtype=np.float32).reshape(IC, 1)
    gamma = np.asarray(inputs["bn_gamma"], dtype=np.float32).reshape(C, 1)
    beta = np.asarray(inputs["bn_beta"], dtype=np.float32).reshape(C, 1)
    # pack all small weights into two tensors (3 DMAs instead of 12)
    wpk = np.ascontiguousarray(
        np.concatenate([dxwT, dywT, gwT, gamma, beta], axis=1))   # [256, 386]
    wpk2 = np.ascontiguousarray(
        np.concatenate([wwT, dxb], axis=1))                        # [128, 257]

    in_maps = []
    for core in range(N_CORES):
        b, h = divmod(core, 2)
        in_maps.append({
            "xl": np.ascontiguousarray(x[b][:, h * NL:(h + 1) * NL]),
            "yl": y[b],
            "wpk": wpk, "wpk2": wpk2,
        })
    return in_maps


def run(inputs, **kw):
    """Run on hardware; returns (full_output, BassKernelResults)."""
    nc = _get_nc()
    in_maps = shard_inputs(inputs)
    r = run_bass_kernel_spmd(nc, in_maps, core_ids=list(range(N_CORES)), **kw)
    out = np.empty((B, C, N), np.float32)
    for core in range(N_CORES):
        b, h = divmod(core, 2)
        out[b][:, h * NL:(h + 1) * NL] = r.results[core]["out"]
    return out.reshape(B, C, HW, HW), r


def kernel(**inputs):
    out, _ = run(inputs)
    return out
